# revision 1
# baseline (speedup 1.0000x reference)
"""Bass/Trainium2 kernel for nn_DDSOpWithReductionOpModel.

Computes out = nonzero(x).sum(dim=0) for x [8192, 8192] fp32 -> [2] int64:
per-row / per-column nonzero counts on device (DVE is_ne mask + accum row
counts; PE ones-matmul PSUM column counts), tiny exact int64 dot on host.

Data-parallel over 8 NeuronCores, rows sharded 1024/core. Streams 2MiB
[128, 4096] sub-tiles on the sync HWDGE queue with a 7-slot buffer pool;
one DVE tensor_scalar pass per sub-tile produces the bf16 mask and the
per-partition row-count accum. Column counts accumulate in PSUM strips
(ones[128,32] stationary, 512-col chunks, 4 banks) and stream out at bank
completion. All engine/queue choices here were A/B-measured on HW; the
optional ACT-assisted split (act_frac) and alternate DMA queues measured
slower or convoy-prone under load, so the defaults stay DVE+sync-only.
"""

import ml_dtypes
import numpy as np

import concourse.bacc as bacc
import concourse.bass as bass
import concourse.mybir as mybir
from concourse.bass_utils import run_bass_kernel_spmd
from concourse.tile import TileContext

N0, N1 = 8192, 8192
N_CORES = 8
R = N0 // N_CORES  # rows per core
CHUNK = 512  # PE column-chunk width (one PSUM bank row)

# per-piece cost estimates (ns) used for engine load balancing
COST_V = 5300.0  # measured under full DMA load: DVE is_ne+accum [128,4096]
COST_A = 9120.0  # measured under full DMA load: ACT square+sign+accum


def tile_plan(rows=R, cols=N1, sub_cols=4096, tail_split=True):
    """Sub-tiling plan: (row_tile, col_start, width) in issue order."""
    nt = rows // 128
    plan = []
    for t in range(nt):
        for s in range(cols // sub_cols):
            plan.append((t, s * sub_cols, sub_cols))
    if tail_split and sub_cols % (4 * CHUNK) == 0:
        t, c0, w = plan.pop()
        plan += [(t, c0, w // 2), (t, c0 + w // 2, w // 4), (t, c0 + 3 * w // 4, w // 4)]
    return plan


def assign_engines(plan, act_frac=None, n_tail_v=3):
    """Greedy least-loaded assignment of pieces to DVE ('v') / ACT ('a').

    The last n_tail_v pieces go to DVE (single-pass latency tail).
    act_frac None -> balance by cost model; 0.0 -> all DVE.
    """
    eng = []
    load = {"v": 0.0, "a": 0.0}
    n = len(plan)
    for i, (t, c0, w) in enumerate(plan):
        if i >= n - n_tail_v or act_frac == 0.0:
            e = "v"
        else:
            cv = (load["v"] + COST_V * w / 4096) / 1.0
            ca = (load["a"] + COST_A * w / 4096) / 1.0
            e = "v" if cv <= ca else "a"
        eng.append(e)
        load[e] += (COST_V if e == "v" else COST_A) * w / 4096
    return eng


def build_nc(
    rows=R,
    cols=N1,
    sub_cols=4096,
    tail_split=True,
    x_bufs=7,
    mask_bufs=5,
    sq_bufs=2,
    act_frac=0.0,
    eng_override=None,
    dma_queues=("sync",),
):
    assert rows % 128 == 0 and cols % CHUNK == 0 and sub_cols % CHUNK == 0
    plan = tile_plan(rows, cols, sub_cols, tail_split)
    # column-wise split of every piece: DVE handles [0:wv), ACT [wv:w).
    # Rates under full DMA load: DVE 1.294 ns/col, ACT 2x1.086 ns/col.
    splits = []
    for i, (t, c0, w) in enumerate(plan):
        if act_frac == 0.0 or w <= 256:
            splits.append(w)
        else:
            wv = int(round(w * 0.75 / 128.0)) * 128
            splits.append(min(w, max(128, wv)))
    eng = ["v" if wv == w else "s" for (t, c0, w), wv in zip(plan, splits)]
    n_s = len(plan)
    n_chunks = cols // CHUNK
    n_banks = (n_chunks + 3) // 4
    assert n_banks <= 8

    touches = []  # (i, j, chunk, bank)
    for i, (t, c0, w) in enumerate(plan):
        for j in range(w // CHUNK):
            ch = (c0 + j * CHUNK) // CHUNK
            touches.append((i, j, ch, ch // 4))
    last_touch = {}
    chunk_first = {}
    chunk_last = {}
    for i, j, ch, b in touches:
        last_touch[b] = (i, j)
        chunk_first.setdefault(ch, (i, j))
        chunk_last[ch] = (i, j)

    has_act = act_frac != 0.0
    nc = bacc.Bacc("TRN2", target_bir_lowering=False)
    x = nc.dram_tensor("x", [rows, cols], mybir.dt.float32, kind="ExternalInput")
    row_cnt_v = nc.dram_tensor(
        "row_cnt_v", [128, n_s], mybir.dt.float32, kind="ExternalOutput"
    )
    row_cnt_a = (
        nc.dram_tensor(
            "row_cnt_a", [128, n_s], mybir.dt.float32, kind="ExternalOutput"
        )
        if has_act
        else None
    )
    col_cnt = nc.dram_tensor(
        "col_cnt", [n_chunks, CHUNK], mybir.dt.float32, kind="ExternalOutput"
    )

    NE = mybir.AluOpType.not_equal
    AF = mybir.ActivationFunctionType

    with TileContext(nc) as tc:
        with (
            tc.tile_pool(name="xp", bufs=x_bufs) as xp,
            tc.tile_pool(name="mp", bufs=mask_bufs) as mp,
            tc.tile_pool(name="sq", bufs=sq_bufs) as sqp,
            tc.tile_pool(name="pp", bufs=1, space="PSUM") as pp,
            tc.tile_pool(name="cp", bufs=1) as cp,
        ):
            ones = cp.tile([128, 32], mybir.dt.bfloat16)
            nc.vector.memset(ones, 1.0)
            rcv = cp.tile([128, n_s], mybir.dt.float32)
            rca = cp.tile([128, n_s], mybir.dt.float32)
            psums = [
                pp.tile([128, CHUNK], mybir.dt.float32, name=f"psum{b}")
                for b in range(n_banks)
            ]
            col_sbs = [
                cp.tile([128, CHUNK], mybir.dt.float32, name=f"colsb{b}")
                for b in range(n_banks)
            ]
            for i, (t, c0, w) in enumerate(plan):
                xt = xp.tile([128, w], mybir.dt.float32, name=f"xt{i}", tag="x")
                qname = dma_queues[i % len(dma_queues)]
                dma_eng = getattr(nc, qname)
                dma_eng.dma_start(out=xt, in_=x[t * 128 : (t + 1) * 128, c0 : c0 + w])
                mt = mp.tile([128, w], mybir.dt.bfloat16, name=f"mt{i}", tag="m")
                wv = splits[i]
                nc.vector.tensor_scalar(
                    out=mt[:, 0:wv],
                    in0=xt[:, 0:wv],
                    scalar1=0.0,
                    scalar2=None,
                    op0=NE,
                    op1=mybir.AluOpType.add,
                    accum_out=rcv[:, i : i + 1],
                )
                if wv < w:
                    wa = w - wv
                    sq = sqp.tile(
                        [128, wa], mybir.dt.bfloat16, name=f"sq{i}", tag="s"
                    )
                    nc.scalar.activation(out=sq, in_=xt[:, wv:w], func=AF.Square)
                    nc.scalar.activation(
                        out=mt[:, wv:w],
                        in_=sq,
                        func=AF.Sign,
                        accum_out=rca[:, i : i + 1],
                    )
                for j in range(w // CHUNK):
                    ch = (c0 + j * CHUNK) // CHUNK
                    b, g = ch // 4, ch % 4
                    nc.tensor.matmul(
                        psums[b][32 * g : 32 * g + 32, :],
                        lhsT=ones,
                        rhs=mt[:, j * CHUNK : (j + 1) * CHUNK],
                        start=(chunk_first[ch] == (i, j)),
                        stop=(chunk_last[ch] == (i, j)),
                        tile_position=(0, 32 * g),
                        skip_group_check=True,
                    )
                    if last_touch[b] == (i, j):
                        nc.vector.tensor_copy(out=col_sbs[b], in_=psums[b])
                        k = min(4, n_chunks - b * 4)
                        nc.sync.dma_start(
                            out=col_cnt[b * 4 : b * 4 + k, :],
                            in_=col_sbs[b][0 : 32 * k : 32, :],
                        )
            if has_act:
                n_bulk = max(0, n_s - 4)
                nc.sync.dma_start(out=row_cnt_v[:, 0:n_bulk], in_=rcv[:, 0:n_bulk])
                nc.sync.dma_start(out=row_cnt_a.ap(), in_=rca)
                nc.scalar.dma_start(
                    out=row_cnt_v[:, n_bulk:n_s], in_=rcv[:, n_bulk:n_s]
                )
            else:
                nc.sync.dma_start(out=row_cnt_v.ap(), in_=rcv)
    nc.compile()
    return nc, plan, eng


_NC_CACHE = {}


def _get_nc():
    if "nc" not in _NC_CACHE:
        _NC_CACHE["nc"] = build_nc()
    return _NC_CACHE["nc"]


def postprocess(results, plan, eng, rows=R, cols=N1):
    """Combine per-core row/col counts into the [2] int64 output."""
    nt = rows // 128
    out_rows = np.int64(0)
    col_counts = np.zeros(cols, dtype=np.int64)
    for core, res in enumerate(results):
        rcp_v = np.rint(np.asarray(res["row_cnt_v"], dtype=np.float64)).astype(np.int64)
        rcp_a = (
            np.rint(np.asarray(res["row_cnt_a"], dtype=np.float64)).astype(np.int64)
            if "row_cnt_a" in res
            else None
        )
        rc = np.zeros((128, nt), dtype=np.int64)
        for i, (t, _, _) in enumerate(plan):
            rc[:, t] += rcp_v[:, i]
            if rcp_a is not None and eng[i] == "s":
                rc[:, t] += rcp_a[:, i]
        local = rc.T.reshape(rows)
        row_idx = np.arange(core * rows, (core + 1) * rows, dtype=np.int64)
        out_rows += np.dot(row_idx, local)
        cc = np.rint(np.asarray(res["col_cnt"], dtype=np.float64)).astype(np.int64)
        col_counts += cc.reshape(cols)
    out_cols = np.dot(np.arange(cols, dtype=np.int64), col_counts)
    return np.array([out_rows, out_cols], dtype=np.int64)


def kernel(inputs, _trace=False, _trace_kwargs=None):
    x = np.ascontiguousarray(np.asarray(inputs, dtype=np.float32))
    assert x.shape == (N0, N1)
    nc, plan, eng = _get_nc()
    in_maps = [{"x": x[c * R : (c + 1) * R]} for c in range(N_CORES)]
    res = run_bass_kernel_spmd(
        nc,
        in_maps,
        core_ids=list(range(N_CORES)),
        trace=_trace,
        **(_trace_kwargs or {}),
    )
    out = postprocess(res.results, plan, eng)
    if _trace:
        return out, res
    return out

